# revision 1
# baseline (speedup 1.0000x reference)
"""Distributed GCN (3x GCNConv + linear head) on 8 TRN2 NeuronCores.

Strategy (graph/data parallel, per sharding hint):
  - Nodes block-sharded across 8 cores (5000 real rows each, padded to 5120).
  - Weights replicated; per-layer: p = H @ W computed locally (feature-major
    H^T in SBUF), cast to fp16, AllGather'ed into a full node-major table
    p_full [40960, 128] fp16 in DRAM (Shared).
  - Edges assigned to the core owning dst. Per core, edges are grouped by
    (dst-window of 128 nodes, src half-table) and padded to 128-edge tiles
    (pad edges gather row 0 with weight 0). The per-edge norm
    dinv[src]*dinv[dst] (and self-loops as explicit edges with dinv^2) is
    folded into a one-hot scatter matrix S built on DVE from uploaded
    per-edge (dst-in-window, norm) arrays:  S = (iota == dst) * norm.
  - Gather: batched nc.gpsimd.dma_gather (int16 idxs) pulls message rows
    M [128 edges, 128 feat] fp16 from p_full; aggregation is
    PSUM += M^T @ S on the PE (feature-major agg window [128f, 128dst]),
    flushed with Relu+bias on ACT straight into next layer's H^T.
  - Head: out = H3 @ lin_w + lin_b via PE + transpose, one [5120] f32 per
    core, host concatenates and trims padding.

Self-contained: hardcodes the problem shapes; all host-side prep derives
from the runtime edge_index only (index bookkeeping + degree).
"""

import os
from contextlib import ExitStack
from dataclasses import dataclass, field

import numpy as np

import concourse.bacc as bacc
import concourse.bass as bass
import concourse.mybir as mybir
import concourse.tile as tile
from concourse.bass_utils import run_bass_kernel_spmd

F32 = mybir.dt.float32
F16 = mybir.dt.float16
I16 = mybir.dt.int16
AF = mybir.ActivationFunctionType
ALU = mybir.AluOpType

D = 128  # feature dim (in = hid = 128)
WIN = 128  # dst nodes per aggregation window
NC = 8  # cores


@dataclass
class Cfg:
    n: int = 40000
    e: int = 640000
    shard: int = 5000  # real nodes per core
    kwin: int = 4  # windows per gather batch
    msg_dtype: object = F16

    @property
    def spad(self):  # padded shard
        return ((self.shard + WIN - 1) // WIN) * WIN

    @property
    def nwin(self):
        return self.spad // WIN

    @property
    def half(self):  # rows per gather half-table
        return NC * self.spad // 2


@dataclass
class Plan:
    """Per-call schedule shared by all cores (static SPMD program)."""

    caps: np.ndarray  # [nwin, 2] tiles per (window, half), max over cores
    tot: int  # total tiles per layer
    chunks: list = field(default_factory=list)
    # chunks: list of dicts:
    #  {"nt": {h: ntiles}, "t0": {h: first-global-tile},
    #   "windows": [(w, [(h, off_in_half_buf, global_tile), ...])]}


def build_plan(caps: np.ndarray, cfg: Cfg) -> Plan:
    plan = Plan(caps=caps, tot=int(caps.sum()))
    t = 0
    for w0 in range(0, cfg.nwin, cfg.kwin):
        ws = list(range(w0, min(w0 + cfg.kwin, cfg.nwin)))
        ch = {"nt": {}, "t0": {}, "windows": []}
        wtiles = {w: [] for w in ws}
        for h in (0, 1):
            ch["t0"][h] = t
            off = 0
            for w in ws:
                for _ in range(int(caps[w, h])):
                    wtiles[w].append((h, off, t))
                    off += 1
                    t += 1
            ch["nt"][h] = off
        ch["windows"] = [(w, wtiles[w]) for w in ws]
        plan.chunks.append(ch)
    assert t == plan.tot
    return plan


def preprocess(edge_index: np.ndarray, cfg: Cfg):
    """Host-side index prep. Returns (plan, per_core dict arrays)."""
    n, shard, spad, half, nwin = cfg.n, cfg.shard, cfg.spad, cfg.half, cfg.nwin
    src = edge_index[0].astype(np.int64)
    dst = edge_index[1].astype(np.int64)
    deg = 1.0 + np.bincount(dst, minlength=n).astype(np.float64)
    dinv = (1.0 / np.sqrt(deg)).astype(np.float32)

    # append self loops as edges
    allsrc = np.concatenate([src, np.arange(n, dtype=np.int64)])
    alldst = np.concatenate([dst, np.arange(n, dtype=np.int64)])
    allnorm = np.concatenate(
        [dinv[src] * dinv[dst], (dinv * dinv).astype(np.float32)]
    ).astype(np.float32)

    core = alldst // shard
    dloc = alldst % shard
    w = dloc // WIN
    dwin = (dloc % WIN).astype(np.float32)
    prow = (allsrc // shard) * spad + (allsrc % shard)
    h = prow // half
    idx = (prow % half).astype(np.int16)
    assert half <= 32768

    # counts per (core, window, half)
    key = (core * nwin + w) * 2 + h
    cnt = np.bincount(key, minlength=NC * nwin * 2).reshape(NC, nwin, 2)
    caps = np.ceil(cnt.max(axis=0) / 128.0).astype(np.int64)  # [nwin, 2]
    caps = np.maximum(caps, 1)
    plan = build_plan(caps, cfg)
    tot = plan.tot

    # order edges per (core, w, h) and place into padded tile stream
    order = np.lexsort((h, w, core))
    osrcidx = idx[order]
    odwin = dwin[order]
    onorm = allnorm[order]
    okey = key[order]
    # boundaries per key
    starts = np.zeros(NC * nwin * 2 + 1, dtype=np.int64)
    np.cumsum(np.bincount(okey, minlength=NC * nwin * 2), out=starts[1:])

    # global slot of each (w,h) group in the tile stream
    gslot = np.zeros((nwin, 2), dtype=np.int64)  # first tile of group
    for ch in plan.chunks:
        for wv, tl in ch["windows"]:
            for hh, _off, gt in tl:
                pass
    # simpler: recompute directly from chunk layout
    for ch in plan.chunks:
        for wv, tl in ch["windows"]:
            firsts = {}
            for hh, _off, gt in tl:
                if hh not in firsts:
                    firsts[hh] = gt
            for hh, gt in firsts.items():
                gslot[wv, hh] = gt

    per_core = []
    for c in range(NC):
        gi = np.zeros(tot * 128, dtype=np.int16)
        dl = np.zeros(tot * 128, dtype=np.float32)
        nv = np.zeros(tot * 128, dtype=np.float32)
        for wv in range(nwin):
            for hh in (0, 1):
                k = (c * nwin + wv) * 2 + hh
                s, e_ = starts[k], starts[k + 1]
                m = e_ - s
                if m == 0:
                    continue
                base = gslot[wv, hh] * 128
                assert m <= caps[wv, hh] * 128
                gi[base : base + m] = osrcidx[s:e_]
                dl[base : base + m] = odwin[s:e_]
                nv[base : base + m] = onorm[s:e_]
        # gather idx layout: idx i -> [i%16 (+16k replicas), i//16]
        gi16 = gi.reshape(tot * 8, 16).T  # [16, tot*8]
        gi128 = np.tile(gi16, (8, 1)).copy()  # [128, tot*8]
        # per-tile scalar columns: [128, tot], entry [e, t] = edge t*128+e
        dl2 = dl.reshape(tot, 128).T.copy()
        nv2 = nv.reshape(tot, 128).T.copy()
        # host-prebuilt scatter one-hots S [tot*128, 128] -> partition-major
        # [128, tot*128]: sflat[e, gt*128+j] = norm of edge (gt,e) if its
        # dst-in-window == j else 0 (pad edges have norm 0)
        sf = np.zeros((tot * 128, 128), np.float16)
        sf[np.arange(tot * 128), dl.astype(np.int64)] = nv.astype(np.float16)
        sflat = np.ascontiguousarray(
            sf.reshape(tot, 128, 128).transpose(1, 0, 2).reshape(128, tot * 128)
        )
        per_core.append(
            {"gidx": gi128, "dstloc": dl2, "normv": nv2, "sflat": sflat}
        )
    return plan, per_core, dinv


def emulate(x, edge_index, Ws, bs, lin_w, lin_b, cfg: Cfg, fp16=True):
    """Numpy emulation of the exact device dataflow (for validation)."""
    plan, per_core, dinv = preprocess(edge_index, cfg)
    spad, nwin, half = cfg.spad, cfg.nwin, cfg.half
    md = np.float16 if fp16 else np.float32
    H = []  # per-core feature-major H^T [128, spad]
    for c in range(NC):
        xs = x[c * cfg.shard : (c + 1) * cfg.shard]
        H.append(
            np.concatenate([xs, np.zeros((spad - cfg.shard, D), np.float32)]).T.copy()
        )
    iota = np.arange(WIN, dtype=np.float32)
    for l in range(3):
        W, b = Ws[l], bs[l]
        pfull = np.zeros((NC * spad, D), md)
        for c in range(NC):
            p = (H[c].T.astype(np.float32) @ W).astype(md)  # [spad, 128]
            pfull[c * spad : (c + 1) * spad] = p
        Hn = []
        for c in range(NC):
            pc = per_core[c]
            HT = np.zeros((D, spad), np.float32)
            for ch in plan.chunks:
                for wv, tl in ch["windows"]:
                    acc = np.zeros((D, WIN), np.float32)
                    for hh, _off, gt in tl:
                        ii = pc["gidx"][:16, gt * 8 : gt * 8 + 8].T.reshape(-1)
                        M = pfull[half * hh + ii.astype(np.int64)]  # [128, D] fp16
                        S = (
                            (iota[None, :] == pc["dstloc"][:, gt : gt + 1])
                            * pc["normv"][:, gt : gt + 1]
                        ).astype(md)
                        acc += (
                            M.astype(np.float32).T @ S.astype(np.float32)
                        )  # [D, WIN]
                    HT[:, wv * WIN : (wv + 1) * WIN] = np.maximum(
                        acc + b[:, None], 0.0
                    )
            Hn.append(HT)
        H = Hn
    out = np.zeros(cfg.n, np.float32)
    for c in range(NC):
        o = H[c].T @ lin_w[:, 0] + lin_b[0]
        out[c * cfg.shard : (c + 1) * cfg.shard] = o[: cfg.shard]
    return out


def build_program(plan: Plan, cfg: Cfg):
    """Build the SPMD Bass program (same NEFF on all 8 cores)."""
    nc = bacc.Bacc("TRN2", target_bir_lowering=False, debug=False, num_devices=NC)
    spad, nwin, half, tot = cfg.spad, cfg.nwin, cfg.half, plan.tot
    MD = cfg.msg_dtype

    # I/O
    xT = nc.dram_tensor("xT", [D, spad], F32, kind="ExternalInput")
    Wd = [
        nc.dram_tensor(f"W{l}", [D, D], F32, kind="ExternalInput") for l in range(3)
    ]
    bd = [
        nc.dram_tensor(f"b{l}", [D, 1], F32, kind="ExternalInput") for l in range(3)
    ]
    linw_d = nc.dram_tensor("lin_w", [D, 1], F32, kind="ExternalInput")
    linb_d = nc.dram_tensor("lin_b", [D, 1], F32, kind="ExternalInput")
    ident_d = nc.dram_tensor("ident", [D, D], F32, kind="ExternalInput")
    gidx_d = nc.dram_tensor("gidx", [D, tot * 8], I16, kind="ExternalInput")
    sflat_d = nc.dram_tensor("sflat", [D, tot * WIN], MD, kind="ExternalInput")
    out_d = nc.dram_tensor("out", [nwin, WIN], F32, kind="ExternalOutput")

    with tile.TileContext(nc) as tc, ExitStack() as stk:
        consts = stk.enter_context(tc.tile_pool(name="consts", bufs=1))
        hpool = stk.enter_context(tc.tile_pool(name="hpool", bufs=2))
        mpool = stk.enter_context(tc.tile_pool(name="mpool", bufs=2))
        spool = stk.enter_context(tc.tile_pool(name="spool", bufs=2))
        pstage = stk.enter_context(tc.tile_pool(name="pstage", bufs=4))
        # PSUM has 8 banks; every PSUM tile slot pads to one bank.
        psum_agg = stk.enter_context(
            tc.tile_pool(name="psum_agg", bufs=4, space="PSUM")
        )
        psum_p = stk.enter_context(tc.tile_pool(name="psum_p", bufs=2, space="PSUM"))
        dram = stk.enter_context(tc.tile_pool(name="dram", bufs=2, space="DRAM"))

        def load_const(name, dr, shape, dtype):
            t = consts.tile(shape, dtype, name=name)
            nc.sync.dma_start(t[:], dr[tuple(slice(0, s) for s in shape)])
            return t

        ident_sb = load_const("ident_sb", ident_d, [D, D], F32)
        W_sb = [load_const(f"W{l}_sb", Wd[l], [D, D], F32) for l in range(3)]
        b_sb = [load_const(f"b{l}_sb", bd[l], [D, 1], F32) for l in range(3)]
        linw_sb = load_const("linw_sb", linw_d, [D, 1], F32)
        linb_sb = load_const("linb_sb", linb_d, [D, 1], F32)
        gidx_sb = load_const("gidx_sb", gidx_d, [D, tot * 8], I16)

        HT = hpool.tile([D, spad], F32, tag="HT", name="HT_x")
        nc.sync.dma_start(HT[:], xT[:, :])

        for l in range(3):
            # ---- p = H @ W_l  (node-major chunks), cast, stage to DRAM ----
            agin = dram.tile([spad, D], MD, tag="agin", name=f"agin{l}")
            for w in range(nwin):
                pp = psum_p.tile([D, D], F32, tag="pp", name=f"pp{l}_{w}")
                nc.tensor.matmul(
                    pp[:],
                    HT[:, w * WIN : (w + 1) * WIN],
                    W_sb[l][:],
                    start=True,
                    stop=True,
                )
                pc = pstage.tile([D, D], MD, tag="pc", name=f"pc{l}_{w}")
                nc.vector.tensor_copy(pc[:], pp[:])
                nc.sync.dma_start(agin[w * WIN : (w + 1) * WIN, :], pc[:])
            pfull = dram.tile(
                [NC * spad, D], MD, tag="pfull", name=f"pfull{l}",
                addr_space="Shared",
            )
            nc.gpsimd.collective_compute(
                "AllGather",
                ALU.bypass,
                replica_groups=[list(range(NC))],
                ins=[agin.opt()],
                outs=[pfull.opt()],
            )

            # ---- edge aggregation ----
            HTn = hpool.tile([D, spad], F32, tag="HT", name=f"HT{l + 1}")
            for ch in plan.chunks:
                mb = {}
                for h in (0, 1):
                    nt = ch["nt"][h]
                    if nt == 0:
                        continue
                    m = mpool.tile(
                        [D, nt, WIN], MD, tag=f"mb{h}", name=f"mb{l}_{ch['t0'][h]}_{h}"
                    )
                    t0 = ch["t0"][h]
                    nc.gpsimd.dma_gather(
                        m[:],
                        pfull[h * half : (h + 1) * half, :],
                        gidx_sb[:, t0 * 8 : (t0 + nt) * 8],
                        nt * 128,
                        nt * 128,
                        D,
                        single_packet=False,
                    )
                    mb[h] = m
                # stream this chunk's prebuilt S tiles (contiguous range)
                sbase = ch["t0"][0]
                scnt = ch["nt"][0] + ch["nt"][1]
                s_sb = spool.tile(
                    [D, scnt * WIN], MD, tag="S", name=f"S{l}_{sbase}"
                )
                nc.sync.dma_start(
                    s_sb[:], sflat_d[:, sbase * WIN : (sbase + scnt) * WIN]
                )
                for wv, tl in ch["windows"]:
                    ap = psum_agg.tile([D, WIN], F32, tag="agg", name=f"agg{l}_{wv}")
                    for i, (hh, off, gt) in enumerate(tl):
                        nc.tensor.matmul(
                            ap[:],
                            mb[hh][:, off, :],
                            s_sb[:, (gt - sbase) * WIN : (gt - sbase + 1) * WIN],
                            start=(i == 0),
                            stop=(i == len(tl) - 1),
                        )
                    nc.scalar.activation(
                        HTn[:, wv * WIN : (wv + 1) * WIN],
                        ap[:],
                        AF.Relu,
                        bias=b_sb[l][:, 0:1],
                    )
            HT = HTn

        # ---- head: out = H3 @ lin_w + lin_b ----
        stage = pstage.tile([D, nwin], F32, tag="stage")
        for w in range(nwin):
            op = psum_p.tile([D, 1], F32, tag="op", name=f"op{w}", bufs=1)
            nc.tensor.matmul(
                op[:], HT[:, w * WIN : (w + 1) * WIN], linw_sb[:, :], start=True,
                stop=True,
            )
            nc.vector.tensor_scalar(
                stage[:, w : w + 1], op[:], linb_sb[:, 0:1], None, op0=ALU.add
            )
        tp = psum_p.tile([nwin, D], F32, tag="tp", bufs=1)
        nc.tensor.transpose(tp[:], stage[:], ident_sb[:])
        ov = pstage.tile([nwin, D], F32, tag="ov")
        nc.vector.tensor_copy(ov[:], tp[:])
        nc.sync.dma_start(out_d[:, :], ov[:])

    nc.compile()
    return nc


LAST = {}


def make_in_maps(inputs, per_core, cfg: Cfg):
    x = np.ascontiguousarray(np.asarray(inputs["x"], dtype=np.float32))
    Ws = [np.asarray(inputs[f"W{l}"], dtype=np.float32) for l in range(3)]
    bs = [np.asarray(inputs[f"b{l}"], dtype=np.float32) for l in range(3)]
    lin_w = np.asarray(inputs["lin_w"], dtype=np.float32)
    lin_b = np.asarray(inputs["lin_b"], dtype=np.float32)
    spad = cfg.spad
    ident = np.eye(D, dtype=np.float32)
    in_maps = []
    for c in range(NC):
        xs = x[c * cfg.shard : (c + 1) * cfg.shard]
        xT = np.zeros((D, spad), np.float32)
        xT[:, : cfg.shard] = xs.T
        im = {
            "xT": xT,
            "lin_w": lin_w.astype(np.float32).reshape(D, 1),
            "lin_b": np.full((D, 1), float(lin_b.reshape(-1)[0]), np.float32),
            "ident": ident,
            "gidx": per_core[c]["gidx"],
            "sflat": per_core[c]["sflat"],
        }
        for l in range(3):
            im[f"W{l}"] = Ws[l]
            im[f"b{l}"] = bs[l].reshape(D, 1)
        in_maps.append(im)
    return in_maps


def kernel(**inputs):
    cfg = Cfg()
    edge_index = np.asarray(inputs["edge_index"], dtype=np.int32)
    plan, per_core, _ = preprocess(edge_index, cfg)
    nc = build_program(plan, cfg)
    in_maps = make_in_maps(inputs, per_core, cfg)

    res = run_bass_kernel_spmd(nc, in_maps, core_ids=list(range(NC)))
    LAST["res"] = res
    out = np.zeros(cfg.n, np.float32)
    for c in range(NC):
        out[c * cfg.shard : (c + 1) * cfg.shard] = res.results[c]["out"].reshape(-1)[
            : cfg.shard
        ]
    return out



# revision 2
# speedup vs baseline: 1.0257x; 1.0257x over previous
"""Distributed GCN (3x GCNConv + linear head) on 8 TRN2 NeuronCores.

Strategy (graph/data parallel, per sharding hint):
  - Nodes block-sharded across 8 cores (5000 real rows each, padded to 5120).
  - Weights replicated; per-layer: p = H @ W computed locally per 128-node
    window (node-major pc tiles kept in SBUF as `plocal`), cast to fp16 and
    DMA'd into two staging buffers agin_A (local rows 0..2559) / agin_B
    (2560..5119). Two AllGathers (A fires as soon as windows 0..19 are done,
    overlapping the rest of the previous layer's aggregation) build two
    shared tables pfull_A/pfull_B [8*2560, 128] fp16 in DRAM.
  - Edges assigned to the core owning dst; self-loop terms are NOT edges:
    they are added per window with one extra matmul against a host-built
    diagonal S (diag(dinv^2)) using the SBUF-resident plocal tiles.
  - Remaining edges are grouped by (dst window of 128 nodes, A/B table) and
    padded to 128-edge tiles. Per-edge norm dinv[src]*dinv[dst] is folded
    into host-built one-hot scatter tiles S [slot, dst-in-window] fp16.
  - Gather: batched nc.gpsimd.dma_gather (int16 idxs < 20480) pulls message
    rows M [128 slots, 128 feat] fp16 from pfull_{A,B}; aggregation is
    PSUM += M^T @ S on the PE, flushed with Relu+bias on ACT into the next
    layer's H^T. The next layer's p-matmul for window w is emitted right
    after window w's flush, so the A-half AllGather of layer l+1 runs while
    layer l's B-half windows are still aggregating.
  - Head: out = H3 @ lin_w + lin_b via PE + transpose, one [5120] f32 per
    core, host concatenates and trims padding.

Self-contained: hardcodes the problem shapes; all host-side prep derives
from the runtime edge_index only (index bookkeeping + degree).
"""

import os
from contextlib import ExitStack
from dataclasses import dataclass, field

import numpy as np

import concourse.bacc as bacc
import concourse.bass as bass
import concourse.mybir as mybir
import concourse.tile as tile
from concourse.bass_utils import run_bass_kernel_spmd

F32 = mybir.dt.float32
F16 = mybir.dt.float16
I16 = mybir.dt.int16
AF = mybir.ActivationFunctionType
ALU = mybir.AluOpType

D = 128  # feature dim (in = hid = 128)
WIN = 128  # dst nodes per aggregation window
NC = 8  # cores


@dataclass
class Cfg:
    n: int = 40000
    e: int = 640000
    shard: int = 5000  # real nodes per core
    kwin: int = 4  # windows per gather batch
    msg_dtype: object = F16

    @property
    def spad(self):  # padded shard
        return ((self.shard + WIN - 1) // WIN) * WIN

    @property
    def nwin(self):
        return self.spad // WIN

    @property
    def hrows(self):  # local rows per A/B table half
        return self.spad // 2

    @property
    def half(self):  # rows per gather table (pfull_A or pfull_B)
        return NC * self.spad // 2


@dataclass
class Plan:
    """Per-call schedule shared by all cores (static SPMD program)."""

    caps: np.ndarray  # [nwin, 2] tiles per (window, half), max over cores
    tot: int  # total tiles per layer
    chunks: list = field(default_factory=list)
    # chunks: list of dicts:
    #  {"nt": {h: ntiles}, "t0": {h: first-global-tile},
    #   "windows": [(w, [(h, off_in_half_buf, global_tile), ...])]}


def build_plan(caps: np.ndarray, cfg: Cfg) -> Plan:
    plan = Plan(caps=caps, tot=int(caps.sum()))
    t = 0
    for w0 in range(0, cfg.nwin, cfg.kwin):
        ws = list(range(w0, min(w0 + cfg.kwin, cfg.nwin)))
        ch = {"nt": {}, "t0": {}, "windows": []}
        wtiles = {w: [] for w in ws}
        for h in (0, 1):
            ch["t0"][h] = t
            off = 0
            for w in ws:
                for _ in range(int(caps[w, h])):
                    wtiles[w].append((h, off, t))
                    off += 1
                    t += 1
            ch["nt"][h] = off
        ch["windows"] = [(w, wtiles[w]) for w in ws]
        plan.chunks.append(ch)
    assert t == plan.tot
    return plan


def preprocess(edge_index: np.ndarray, cfg: Cfg):
    """Host-side index prep. Returns (plan, per_core dict arrays, dinv)."""
    n, shard, spad, hrows, nwin = cfg.n, cfg.shard, cfg.spad, cfg.hrows, cfg.nwin
    src = edge_index[0].astype(np.int64)
    dst = edge_index[1].astype(np.int64)
    deg = 1.0 + np.bincount(dst, minlength=n).astype(np.float64)
    dinv = (1.0 / np.sqrt(deg)).astype(np.float32)

    allnorm = (dinv[src] * dinv[dst]).astype(np.float32)

    core = dst // shard
    dloc = dst % shard
    w = dloc // WIN
    dwin = (dloc % WIN).astype(np.float32)
    sc = src // shard
    sloc = src % shard
    h = sloc // hrows  # 0 = table A (local rows < hrows), 1 = table B
    idx = (sc * hrows + (sloc % hrows)).astype(np.int16)
    assert NC * hrows <= 32768

    # counts per (core, window, half)
    key = (core * nwin + w) * 2 + h
    cnt = np.bincount(key, minlength=NC * nwin * 2).reshape(NC, nwin, 2)
    caps = np.ceil(cnt.max(axis=0) / 128.0).astype(np.int64)  # [nwin, 2]
    caps = np.maximum(caps, 1)
    plan = build_plan(caps, cfg)
    tot = plan.tot

    # order edges per (core, w, h) and place into padded tile stream
    order = np.lexsort((h, w, core))
    osrcidx = idx[order]
    odwin = dwin[order]
    onorm = allnorm[order]
    okey = key[order]
    starts = np.zeros(NC * nwin * 2 + 1, dtype=np.int64)
    np.cumsum(np.bincount(okey, minlength=NC * nwin * 2), out=starts[1:])

    # global slot of each (w,h) group in the tile stream
    gslot = np.zeros((nwin, 2), dtype=np.int64)
    for ch in plan.chunks:
        for wv, tl in ch["windows"]:
            firsts = {}
            for hh, _off, gt in tl:
                if hh not in firsts:
                    firsts[hh] = gt
            for hh, gt in firsts.items():
                gslot[wv, hh] = gt

    per_core = []
    for c in range(NC):
        gi = np.zeros(tot * 128, dtype=np.int16)
        dl = np.zeros(tot * 128, dtype=np.float32)
        nv = np.zeros(tot * 128, dtype=np.float32)
        for wv in range(nwin):
            for hh in (0, 1):
                k = (c * nwin + wv) * 2 + hh
                s, e_ = starts[k], starts[k + 1]
                m = e_ - s
                if m == 0:
                    continue
                base = gslot[wv, hh] * 128
                assert m <= caps[wv, hh] * 128
                gi[base : base + m] = osrcidx[s:e_]
                dl[base : base + m] = odwin[s:e_]
                nv[base : base + m] = onorm[s:e_]
        # gather idx layout: idx i -> [i%16 (+16k replicas), i//16]
        gi16 = gi.reshape(tot * 8, 16).T  # [16, tot*8]
        gi128 = np.tile(gi16, (8, 1)).copy()  # [128, tot*8]
        # host-prebuilt scatter one-hots S [tot*128, 128] -> partition-major
        sf = np.zeros((tot * 128, 128), np.float16)
        sf[np.arange(tot * 128), dl.astype(np.int64)] = nv.astype(np.float16)
        sflat = np.ascontiguousarray(
            sf.reshape(tot, 128, 128).transpose(1, 0, 2).reshape(128, tot * 128)
        )
        # self-loop diagonal S per window: diag(dinv^2) over local rows
        dg = np.zeros((128, nwin * 128), np.float16)
        for wv in range(nwin):
            rows = np.arange(wv * 128, (wv + 1) * 128) + c * shard
            val = np.where(
                np.arange(wv * 128, (wv + 1) * 128) < shard,
                (dinv[np.minimum(rows, n - 1)] ** 2),
                0.0,
            ).astype(np.float16)
            dg[np.arange(128), wv * 128 + np.arange(128)] = val
        per_core.append({"gidx": gi128, "sflat": sflat, "diag": dg})
    return plan, per_core, dinv


def emulate(x, edge_index, Ws, bs, lin_w, lin_b, cfg: Cfg, fp16=True):
    """Numpy emulation of the exact device dataflow (for validation)."""
    plan, per_core, dinv = preprocess(edge_index, cfg)
    spad, nwin, hrows = cfg.spad, cfg.nwin, cfg.hrows
    md = np.float16 if fp16 else np.float32
    H = []  # per-core feature-major H^T [128, spad]
    for c in range(NC):
        xs = x[c * cfg.shard : (c + 1) * cfg.shard]
        H.append(
            np.concatenate([xs, np.zeros((spad - cfg.shard, D), np.float32)]).T.copy()
        )
    for l in range(3):
        W, b = Ws[l], bs[l]
        pf = [np.zeros((NC * hrows, D), md) for _ in range(2)]
        plocal = []
        for c in range(NC):
            p = (H[c].T.astype(np.float32) @ W).astype(md)  # [spad, 128]
            pf[0][c * hrows : (c + 1) * hrows] = p[:hrows]
            pf[1][c * hrows : (c + 1) * hrows] = p[hrows:]
            plocal.append(p)
        Hn = []
        for c in range(NC):
            pc = per_core[c]
            HT = np.zeros((D, spad), np.float32)
            for ch in plan.chunks:
                for wv, tl in ch["windows"]:
                    acc = np.zeros((D, WIN), np.float32)
                    for hh, _off, gt in tl:
                        ii = pc["gidx"][:16, gt * 8 : gt * 8 + 8].T.reshape(-1)
                        M = pf[hh][ii.astype(np.int64)]  # [128, D]
                        S = pc["sflat"][:, gt * 128 : (gt + 1) * 128]
                        acc += M.astype(np.float32).T @ S.astype(np.float32)
                    # self-loop diag matmul
                    Mw = plocal[c][wv * 128 : (wv + 1) * 128]  # [128, D]
                    Sd = pc["diag"][:, wv * 128 : (wv + 1) * 128]
                    acc += Mw.astype(np.float32).T @ Sd.astype(np.float32)
                    HT[:, wv * WIN : (wv + 1) * WIN] = np.maximum(
                        acc + b[:, None], 0.0
                    )
            Hn.append(HT)
        H = Hn
    out = np.zeros(cfg.n, np.float32)
    for c in range(NC):
        o = H[c].T @ lin_w[:, 0] + lin_b[0]
        out[c * cfg.shard : (c + 1) * cfg.shard] = o[: cfg.shard]
    return out


def build_program(plan: Plan, cfg: Cfg):
    """Build the SPMD Bass program (same NEFF on all 8 cores)."""
    nc = bacc.Bacc("TRN2", target_bir_lowering=False, debug=False, num_devices=NC)
    spad, nwin, hrows, half, tot = cfg.spad, cfg.nwin, cfg.hrows, cfg.half, plan.tot
    MD = cfg.msg_dtype
    nA = nwin // 2  # windows in table A

    xT = nc.dram_tensor("xT", [D, spad], F32, kind="ExternalInput")
    Wd = [
        nc.dram_tensor(f"W{l}", [D, D], F32, kind="ExternalInput") for l in range(3)
    ]
    bd = [
        nc.dram_tensor(f"b{l}", [D, 1], F32, kind="ExternalInput") for l in range(3)
    ]
    linw_d = nc.dram_tensor("lin_w", [D, 1], F32, kind="ExternalInput")
    linb_d = nc.dram_tensor("lin_b", [D, 1], F32, kind="ExternalInput")
    ident_d = nc.dram_tensor("ident", [D, D], F32, kind="ExternalInput")
    gidx_d = nc.dram_tensor("gidx", [D, tot * 8], I16, kind="ExternalInput")
    sflat_d = nc.dram_tensor("sflat", [D, tot * WIN], MD, kind="ExternalInput")
    diag_d = nc.dram_tensor("diag", [D, nwin * WIN], MD, kind="ExternalInput")
    out_d = nc.dram_tensor("out", [nwin, WIN], F32, kind="ExternalOutput")

    with tile.TileContext(nc) as tc, ExitStack() as stk:
        consts = stk.enter_context(tc.tile_pool(name="consts", bufs=1))
        hpool = stk.enter_context(tc.tile_pool(name="hpool", bufs=2))
        ppool = stk.enter_context(tc.tile_pool(name="ppool", bufs=2))
        mpool = stk.enter_context(tc.tile_pool(name="mpool", bufs=2))
        spool = stk.enter_context(tc.tile_pool(name="spool", bufs=2))
        pstage = stk.enter_context(tc.tile_pool(name="pstage", bufs=2))
        psum_agg = stk.enter_context(
            tc.tile_pool(name="psum_agg", bufs=4, space="PSUM")
        )
        psum_p = stk.enter_context(tc.tile_pool(name="psum_p", bufs=2, space="PSUM"))
        dram = stk.enter_context(tc.tile_pool(name="dram", bufs=2, space="DRAM"))

        def load_const(name, dr, shape, dtype):
            t = consts.tile(shape, dtype, name=name)
            nc.sync.dma_start(t[:], dr[tuple(slice(0, s) for s in shape)])
            return t

        ident_sb = load_const("ident_sb", ident_d, [D, D], F32)
        W_sb = [load_const(f"W{l}_sb", Wd[l], [D, D], F32) for l in range(3)]
        b_sb = [load_const(f"b{l}_sb", bd[l], [D, 1], F32) for l in range(3)]
        linw_sb = load_const("linw_sb", linw_d, [D, 1], F32)
        linb_sb = load_const("linb_sb", linb_d, [D, 1], F32)
        gidx_sb = load_const("gidx_sb", gidx_d, [D, tot * 8], I16)
        diag_sb = load_const("diag_sb", diag_d, [D, nwin * WIN], MD)

        def new_ptables(l):
            agin = [
                dram.tile([hrows, D], MD, tag=f"agin{ab}", name=f"agin{ab}{l}")
                for ab in "AB"
            ]
            pfull = [
                dram.tile(
                    [half, D], MD, tag=f"pfull{ab}", name=f"pfull{ab}{l}",
                    addr_space="Shared",
                )
                for ab in "AB"
            ]
            plocal = ppool.tile([D, nwin, D], MD, tag="plocal", name=f"plocal{l}")
            return agin, pfull, plocal

        def emit_pmm(HTsrc, l, w, agin, plocal):
            """p = H[:, w] @ W_l, cast fp16, into plocal + agin half."""
            pp = psum_p.tile([D, D], F32, tag="pp", name=f"pp{l}_{w}")
            nc.tensor.matmul(
                pp[:], HTsrc[:, w * WIN : (w + 1) * WIN], W_sb[l][:],
                start=True, stop=True,
            )
            nc.vector.tensor_copy(plocal[:, w, :], pp[:])
            hh, wl = (0, w) if w < nA else (1, w - nA)
            nc.sync.dma_start(
                agin[hh][wl * WIN : (wl + 1) * WIN, :], plocal[:, w, :]
            )

        def emit_ag(agin, pfull, hh, l):
            nc.gpsimd.collective_compute(
                "AllGather",
                ALU.bypass,
                replica_groups=[list(range(NC))],
                ins=[agin[hh].opt()],
                outs=[pfull[hh].opt()],
            )

        # ---- prologue: load x, p-mms for layer 0, AGs ----
        HT = hpool.tile([D, spad], F32, tag="HT", name="HT_x")
        nc.sync.dma_start(HT[:], xT[:, :])
        agin, pfull, plocal = new_ptables(0)
        for w in range(nwin):
            emit_pmm(HT, 0, w, agin, plocal)
            if w == nA - 1:
                emit_ag(agin, pfull, 0, 0)
        emit_ag(agin, pfull, 1, 0)

        for l in range(3):
            last = l == 2
            if not last:
                agin_n, pfull_n, plocal_n = new_ptables(l + 1)
            HTn = hpool.tile([D, spad], F32, tag="HT", name=f"HT{l + 1}")
            for ch in plan.chunks:
                mb = {}
                for h in (0, 1):
                    nt = ch["nt"][h]
                    if nt == 0:
                        continue
                    m = mpool.tile(
                        [D, nt, WIN], MD, tag=f"mb{h}",
                        name=f"mb{l}_{ch['t0'][h]}_{h}",
                    )
                    t0 = ch["t0"][h]
                    nc.gpsimd.dma_gather(
                        m[:],
                        pfull[h][:, :],
                        gidx_sb[:, t0 * 8 : (t0 + nt) * 8],
                        nt * 128,
                        nt * 128,
                        D,
                        single_packet=False,
                    )
                    mb[h] = m
                sbase = ch["t0"][0]
                scnt = ch["nt"][0] + ch["nt"][1]
                s_sb = spool.tile(
                    [D, scnt * WIN], MD, tag="S", name=f"S{l}_{sbase}"
                )
                nc.sync.dma_start(
                    s_sb[:], sflat_d[:, sbase * WIN : (sbase + scnt) * WIN]
                )
                for wv, tl in ch["windows"]:
                    ap = psum_agg.tile([D, WIN], F32, tag="agg", name=f"agg{l}_{wv}")
                    for i, (hh, off, gt) in enumerate(tl):
                        nc.tensor.matmul(
                            ap[:],
                            mb[hh][:, off, :],
                            s_sb[:, (gt - sbase) * WIN : (gt - sbase + 1) * WIN],
                            start=(i == 0),
                            stop=False,
                        )
                    # self-loop term: p_local window against diag(dinv^2)
                    nc.tensor.matmul(
                        ap[:],
                        plocal[:, wv, :],
                        diag_sb[:, wv * WIN : (wv + 1) * WIN],
                        start=False,
                        stop=True,
                    )
                    nc.scalar.activation(
                        HTn[:, wv * WIN : (wv + 1) * WIN],
                        ap[:],
                        AF.Relu,
                        bias=b_sb[l][:, 0:1],
                    )
                    if not last:
                        emit_pmm(HTn, l + 1, wv, agin_n, plocal_n)
                        if wv == nA - 1:
                            emit_ag(agin_n, pfull_n, 0, l + 1)
            if not last:
                emit_ag(agin_n, pfull_n, 1, l + 1)
                agin, pfull, plocal = agin_n, pfull_n, plocal_n
            HT = HTn

        # ---- head: out = H3 @ lin_w + lin_b ----
        stage = pstage.tile([D, nwin], F32, tag="stage")
        for w in range(nwin):
            op = psum_p.tile([D, 1], F32, tag="op", name=f"op{w}", bufs=1)
            nc.tensor.matmul(
                op[:], HT[:, w * WIN : (w + 1) * WIN], linw_sb[:, :], start=True,
                stop=True,
            )
            nc.vector.tensor_scalar(
                stage[:, w : w + 1], op[:], linb_sb[:, 0:1], None, op0=ALU.add
            )
        tp = psum_p.tile([nwin, D], F32, tag="tp", bufs=1)
        nc.tensor.transpose(tp[:], stage[:], ident_sb[:])
        ov = pstage.tile([nwin, D], F32, tag="ov")
        nc.vector.tensor_copy(ov[:], tp[:])
        nc.sync.dma_start(out_d[:, :], ov[:])

    nc.compile()
    return nc


LAST = {}


def make_in_maps(inputs, per_core, cfg: Cfg):
    x = np.ascontiguousarray(np.asarray(inputs["x"], dtype=np.float32))
    Ws = [np.asarray(inputs[f"W{l}"], dtype=np.float32) for l in range(3)]
    bs = [np.asarray(inputs[f"b{l}"], dtype=np.float32) for l in range(3)]
    lin_w = np.asarray(inputs["lin_w"], dtype=np.float32)
    lin_b = np.asarray(inputs["lin_b"], dtype=np.float32)
    spad = cfg.spad
    ident = np.eye(D, dtype=np.float32)
    in_maps = []
    for c in range(NC):
        xs = x[c * cfg.shard : (c + 1) * cfg.shard]
        xT = np.zeros((D, spad), np.float32)
        xT[:, : cfg.shard] = xs.T
        im = {
            "xT": xT,
            "lin_w": lin_w.astype(np.float32).reshape(D, 1),
            "lin_b": np.full((D, 1), float(lin_b.reshape(-1)[0]), np.float32),
            "ident": ident,
            "gidx": per_core[c]["gidx"],
            "sflat": per_core[c]["sflat"],
            "diag": per_core[c]["diag"],
        }
        for l in range(3):
            im[f"W{l}"] = Ws[l]
            im[f"b{l}"] = bs[l].reshape(D, 1)
        in_maps.append(im)
    return in_maps


def kernel(**inputs):
    cfg = Cfg()
    edge_index = np.asarray(inputs["edge_index"], dtype=np.int32)
    plan, per_core, _ = preprocess(edge_index, cfg)
    nc = build_program(plan, cfg)
    in_maps = make_in_maps(inputs, per_core, cfg)

    res = run_bass_kernel_spmd(nc, in_maps, core_ids=list(range(NC)))
    LAST["res"] = res
    out = np.zeros(cfg.n, np.float32)
    for c in range(NC):
        out[c * cfg.shard : (c + 1) * cfg.shard] = res.results[c]["out"].reshape(-1)[
            : cfg.shard
        ]
    return out


# revision 3
# speedup vs baseline: 1.9428x; 1.8942x over previous
"""Distributed GCN (3x GCNConv + linear head) on 8 TRN2 NeuronCores.

Strategy (graph/data parallel, per sharding hint):
  - Nodes block-sharded across 8 cores (5000 real rows each, padded to 5120).
  - Weights replicated; per-layer: p = H @ W computed locally per 128-node
    window (node-major pc tiles kept in SBUF as `plocal`), cast to fp16 and
    DMA'd into two staging buffers agin_A (local rows 0..2559) / agin_B
    (2560..5119). Two AllGathers (A fires as soon as windows 0..19 are done,
    overlapping the rest of the previous layer's aggregation) build two
    shared tables pfull_A/pfull_B [8*2560, 128] fp16 in DRAM.
  - Edges assigned to the core owning dst; self-loop terms are NOT edges:
    they are added per window with one extra matmul against a host-built
    diagonal S (diag(dinv^2)) using the SBUF-resident plocal tiles.
  - Remaining edges are grouped by (dst window of 128 nodes, A/B table) and
    padded to 128-edge tiles. Per-edge norm dinv[src]*dinv[dst] is folded
    into host-built one-hot scatter tiles S [slot, dst-in-window] fp16.
  - Gather: batched nc.gpsimd.dma_gather (int16 idxs < 20480) pulls message
    rows M [128 slots, 128 feat] fp16 from pfull_{A,B}; aggregation is
    PSUM += M^T @ S on the PE, flushed with Relu+bias on ACT into the next
    layer's H^T. The next layer's p-matmul for window w is emitted right
    after window w's flush, so the A-half AllGather of layer l+1 runs while
    layer l's B-half windows are still aggregating.
  - Head: out = H3 @ lin_w + lin_b via PE + transpose, one [5120] f32 per
    core, host concatenates and trims padding.

Self-contained: hardcodes the problem shapes; all host-side prep derives
from the runtime edge_index only (index bookkeeping + degree).
"""

import os
from contextlib import ExitStack
from dataclasses import dataclass, field

import numpy as np

import concourse.bacc as bacc
import concourse.bass as bass
import concourse.mybir as mybir
import concourse.tile as tile
from concourse.bass_utils import run_bass_kernel_spmd

F32 = mybir.dt.float32
F16 = mybir.dt.float16
I16 = mybir.dt.int16
AF = mybir.ActivationFunctionType
ALU = mybir.AluOpType

D = 128  # feature dim (in = hid = 128)
WIN = 128  # dst nodes per aggregation window
NC = 8  # cores


@dataclass
class Cfg:
    n: int = 40000
    e: int = 640000
    shard: int = 5000  # real nodes per core
    kwin: int = 4  # windows per gather batch
    msg_dtype: object = F16

    @property
    def spad(self):  # padded shard
        return ((self.shard + WIN - 1) // WIN) * WIN

    @property
    def nwin(self):
        return self.spad // WIN

    @property
    def hrows(self):  # local rows per A/B table half
        return self.spad // 2

    @property
    def half(self):  # rows per gather table (pfull_A or pfull_B)
        return NC * self.spad // 2


@dataclass
class Plan:
    """Per-call schedule shared by all cores (static SPMD program)."""

    caps: np.ndarray  # [nwin, 2] tiles per (window, half), max over cores
    tot: int  # total tiles per layer
    chunks: list = field(default_factory=list)
    # chunks: list of dicts:
    #  {"nt": {h: ntiles}, "t0": {h: first-global-tile},
    #   "windows": [(w, [(h, off_in_half_buf, global_tile), ...])]}


def build_plan(caps: np.ndarray, cfg: Cfg) -> Plan:
    plan = Plan(caps=caps, tot=int(caps.sum()))
    t = 0
    for w0 in range(0, cfg.nwin, cfg.kwin):
        ws = list(range(w0, min(w0 + cfg.kwin, cfg.nwin)))
        ch = {"nt": {}, "t0": {}, "windows": []}
        wtiles = {w: [] for w in ws}
        for h in (0, 1):
            ch["t0"][h] = t
            off = 0
            for w in ws:
                for _ in range(int(caps[w, h])):
                    wtiles[w].append((h, off, t))
                    off += 1
                    t += 1
            ch["nt"][h] = off
        ch["windows"] = [(w, wtiles[w]) for w in ws]
        plan.chunks.append(ch)
    assert t == plan.tot
    return plan


def preprocess(edge_index: np.ndarray, cfg: Cfg):
    """Host-side index prep. Returns (plan, per_core dict arrays, dinv)."""
    n, shard, spad, hrows, nwin = cfg.n, cfg.shard, cfg.spad, cfg.hrows, cfg.nwin
    src = edge_index[0].astype(np.int64)
    dst = edge_index[1].astype(np.int64)
    deg = 1.0 + np.bincount(dst, minlength=n).astype(np.float64)
    dinv = (1.0 / np.sqrt(deg)).astype(np.float32)

    allnorm = (dinv[src] * dinv[dst]).astype(np.float32)

    core = dst // shard
    dloc = dst % shard
    w = dloc // WIN
    dwin = (dloc % WIN).astype(np.float32)
    sc = src // shard
    sloc = src % shard
    h = sloc // hrows  # 0 = table A (local rows < hrows), 1 = table B
    idx = (sc * hrows + (sloc % hrows)).astype(np.int16)
    assert NC * hrows <= 32768

    # counts per (core, window, half)
    key = (core * nwin + w) * 2 + h
    cnt = np.bincount(key, minlength=NC * nwin * 2).reshape(NC, nwin, 2)
    caps = np.ceil(cnt.max(axis=0) / 128.0).astype(np.int64)  # [nwin, 2]
    caps = np.maximum(caps, 1)
    plan = build_plan(caps, cfg)
    tot = plan.tot

    # order edges per (core, w, h) and place into padded tile stream
    order = np.lexsort((h, w, core))
    osrcidx = idx[order]
    odwin = dwin[order]
    onorm = allnorm[order]
    okey = key[order]
    starts = np.zeros(NC * nwin * 2 + 1, dtype=np.int64)
    np.cumsum(np.bincount(okey, minlength=NC * nwin * 2), out=starts[1:])

    # global slot of each (w,h) group in the tile stream
    gslot = np.zeros((nwin, 2), dtype=np.int64)
    for ch in plan.chunks:
        for wv, tl in ch["windows"]:
            firsts = {}
            for hh, _off, gt in tl:
                if hh not in firsts:
                    firsts[hh] = gt
            for hh, gt in firsts.items():
                gslot[wv, hh] = gt

    per_core = []
    for c in range(NC):
        gi = np.zeros(tot * 128, dtype=np.int16)
        dl = np.zeros(tot * 128, dtype=np.float32)
        nv = np.zeros(tot * 128, dtype=np.float32)
        for wv in range(nwin):
            for hh in (0, 1):
                k = (c * nwin + wv) * 2 + hh
                s, e_ = starts[k], starts[k + 1]
                m = e_ - s
                if m == 0:
                    continue
                base = gslot[wv, hh] * 128
                assert m <= caps[wv, hh] * 128
                gi[base : base + m] = osrcidx[s:e_]
                dl[base : base + m] = odwin[s:e_]
                nv[base : base + m] = onorm[s:e_]
        # gather idx layout: idx i -> [i%16 (+16k replicas), i//16]
        gi16 = gi.reshape(tot * 8, 16).T  # [16, tot*8]
        gi128 = np.tile(gi16, (8, 1)).copy()  # [128, tot*8]
        # host-prebuilt scatter one-hots S [tot*128, 128] -> partition-major
        sf = np.zeros((tot * 128, 128), np.float16)
        sf[np.arange(tot * 128), dl.astype(np.int64)] = nv.astype(np.float16)
        sflat = np.ascontiguousarray(
            sf.reshape(tot, 128, 128).transpose(1, 0, 2).reshape(128, tot * 128)
        )
        # self-loop diagonal S per window: diag(dinv^2) over local rows
        dg = np.zeros((128, nwin * 128), np.float16)
        for wv in range(nwin):
            rows = np.arange(wv * 128, (wv + 1) * 128) + c * shard
            val = np.where(
                np.arange(wv * 128, (wv + 1) * 128) < shard,
                (dinv[np.minimum(rows, n - 1)] ** 2),
                0.0,
            ).astype(np.float16)
            dg[np.arange(128), wv * 128 + np.arange(128)] = val
        per_core.append({"gidx": gi128, "sflat": sflat, "diag": dg})
    return plan, per_core, dinv


def emulate(x, edge_index, Ws, bs, lin_w, lin_b, cfg: Cfg, fp16=True):
    """Numpy emulation of the exact device dataflow (for validation)."""
    plan, per_core, dinv = preprocess(edge_index, cfg)
    spad, nwin, hrows = cfg.spad, cfg.nwin, cfg.hrows
    md = np.float16 if fp16 else np.float32
    H = []  # per-core feature-major H^T [128, spad]
    for c in range(NC):
        xs = x[c * cfg.shard : (c + 1) * cfg.shard]
        H.append(
            np.concatenate([xs, np.zeros((spad - cfg.shard, D), np.float32)]).T.copy()
        )
    for l in range(3):
        W, b = Ws[l], bs[l]
        pf = [np.zeros((NC * hrows, D), md) for _ in range(2)]
        plocal = []
        for c in range(NC):
            p = (H[c].T.astype(np.float32) @ W).astype(md)  # [spad, 128]
            pf[0][c * hrows : (c + 1) * hrows] = p[:hrows]
            pf[1][c * hrows : (c + 1) * hrows] = p[hrows:]
            plocal.append(p)
        Hn = []
        for c in range(NC):
            pc = per_core[c]
            HT = np.zeros((D, spad), np.float32)
            for ch in plan.chunks:
                for wv, tl in ch["windows"]:
                    acc = np.zeros((D, WIN), np.float32)
                    for hh, _off, gt in tl:
                        ii = pc["gidx"][:16, gt * 8 : gt * 8 + 8].T.reshape(-1)
                        M = pf[hh][ii.astype(np.int64)]  # [128, D]
                        S = pc["sflat"][:, gt * 128 : (gt + 1) * 128]
                        acc += M.astype(np.float32).T @ S.astype(np.float32)
                    # self-loop diag matmul
                    Mw = plocal[c][wv * 128 : (wv + 1) * 128]  # [128, D]
                    Sd = pc["diag"][:, wv * 128 : (wv + 1) * 128]
                    acc += Mw.astype(np.float32).T @ Sd.astype(np.float32)
                    HT[:, wv * WIN : (wv + 1) * WIN] = np.maximum(
                        acc + b[:, None], 0.0
                    )
            Hn.append(HT)
        H = Hn
    out = np.zeros(cfg.n, np.float32)
    for c in range(NC):
        o = H[c].T @ lin_w[:, 0] + lin_b[0]
        out[c * cfg.shard : (c + 1) * cfg.shard] = o[: cfg.shard]
    return out


def build_program(plan: Plan, cfg: Cfg):
    """Build the SPMD Bass program (same NEFF on all 8 cores)."""
    nc = bacc.Bacc("TRN2", target_bir_lowering=False, debug=False, num_devices=NC,
                   num_swdge_queues=4, dynamic_dma_scratch_size=32768)
    spad, nwin, hrows, half, tot = cfg.spad, cfg.nwin, cfg.hrows, cfg.half, plan.tot
    MD = cfg.msg_dtype
    nA = nwin // 2  # windows in table A

    xT = nc.dram_tensor("xT", [D, spad], F32, kind="ExternalInput")
    Wd = [
        nc.dram_tensor(f"W{l}", [D, D], F32, kind="ExternalInput") for l in range(3)
    ]
    bd = [
        nc.dram_tensor(f"b{l}", [D, 1], F32, kind="ExternalInput") for l in range(3)
    ]
    linw_d = nc.dram_tensor("lin_w", [D, 1], F32, kind="ExternalInput")
    linb_d = nc.dram_tensor("lin_b", [D, 1], F32, kind="ExternalInput")
    ident_d = nc.dram_tensor("ident", [D, D], F32, kind="ExternalInput")
    gidx_d = nc.dram_tensor("gidx", [D, tot * 8], I16, kind="ExternalInput")
    sflat_d = nc.dram_tensor("sflat", [D, tot * WIN], MD, kind="ExternalInput")
    diag_d = nc.dram_tensor("diag", [D, nwin * WIN], MD, kind="ExternalInput")
    out_d = nc.dram_tensor("out", [nwin, WIN], F32, kind="ExternalOutput")

    with tile.TileContext(nc) as tc, ExitStack() as stk:
        consts = stk.enter_context(tc.tile_pool(name="consts", bufs=1))
        hpool = stk.enter_context(tc.tile_pool(name="hpool", bufs=2))
        ppool = stk.enter_context(tc.tile_pool(name="ppool", bufs=2))
        mpool = stk.enter_context(tc.tile_pool(name="mpool", bufs=2))
        spool = stk.enter_context(tc.tile_pool(name="spool", bufs=2))
        pstage = stk.enter_context(tc.tile_pool(name="pstage", bufs=2))
        psum_agg = stk.enter_context(
            tc.tile_pool(name="psum_agg", bufs=4, space="PSUM")
        )
        psum_p = stk.enter_context(tc.tile_pool(name="psum_p", bufs=2, space="PSUM"))
        dram = stk.enter_context(tc.tile_pool(name="dram", bufs=2, space="DRAM"))

        def load_const(name, dr, shape, dtype):
            t = consts.tile(shape, dtype, name=name)
            nc.sync.dma_start(t[:], dr[tuple(slice(0, s) for s in shape)])
            return t

        ident_sb = load_const("ident_sb", ident_d, [D, D], F32)
        W_sb = [load_const(f"W{l}_sb", Wd[l], [D, D], F32) for l in range(3)]
        b_sb = [load_const(f"b{l}_sb", bd[l], [D, 1], F32) for l in range(3)]
        linw_sb = load_const("linw_sb", linw_d, [D, 1], F32)
        linb_sb = load_const("linb_sb", linb_d, [D, 1], F32)
        gidx_sb = load_const("gidx_sb", gidx_d, [D, tot * 8], I16)
        diag_sb = load_const("diag_sb", diag_d, [D, nwin * WIN], MD)

        def new_ptables(l):
            agin = [
                dram.tile([hrows, D], MD, tag=f"agin{ab}", name=f"agin{ab}{l}")
                for ab in "AB"
            ]
            pfull = [
                dram.tile(
                    [half, D], MD, tag=f"pfull{ab}", name=f"pfull{ab}{l}",
                    addr_space="Shared",
                )
                for ab in "AB"
            ]
            plocal = ppool.tile([D, nwin, D], MD, tag="plocal", name=f"plocal{l}")
            return agin, pfull, plocal

        def emit_pmm(HTsrc, l, w, agin, plocal):
            """p = H[:, w] @ W_l, cast fp16, into plocal + agin half."""
            pp = psum_p.tile([D, D], F32, tag="pp", name=f"pp{l}_{w}")
            nc.tensor.matmul(
                pp[:], HTsrc[:, w * WIN : (w + 1) * WIN], W_sb[l][:],
                start=True, stop=True,
            )
            nc.vector.tensor_copy(plocal[:, w, :], pp[:])
            hh, wl = (0, w) if w < nA else (1, w - nA)
            nc.sync.dma_start(
                agin[hh][wl * WIN : (wl + 1) * WIN, :], plocal[:, w, :]
            )

        def emit_ag(agin, pfull, hh, l):
            nc.gpsimd.collective_compute(
                "AllGather",
                ALU.bypass,
                replica_groups=[list(range(NC))],
                ins=[agin[hh].opt()],
                outs=[pfull[hh].opt()],
            )

        # ---- prologue: load x, p-mms for layer 0, AGs ----
        HT = hpool.tile([D, spad], F32, tag="HT", name="HT_x")
        nc.sync.dma_start(HT[:], xT[:, :])
        agin, pfull, plocal = new_ptables(0)
        for w in range(nwin):
            emit_pmm(HT, 0, w, agin, plocal)
            if w == nA - 1:
                emit_ag(agin, pfull, 0, 0)
        emit_ag(agin, pfull, 1, 0)

        for l in range(3):
            last = l == 2
            if not last:
                agin_n, pfull_n, plocal_n = new_ptables(l + 1)
            HTn = hpool.tile([D, spad], F32, tag="HT", name=f"HT{l + 1}")
            for ci, ch in enumerate(plan.chunks):
                mb = {}
                for h in (0, 1):
                    nt = ch["nt"][h]
                    if nt == 0:
                        continue
                    m = mpool.tile(
                        [D, nt, WIN], MD, tag=f"mb{h}",
                        name=f"mb{l}_{ch['t0'][h]}_{h}",
                    )
                    t0 = ch["t0"][h]
                    nc.gpsimd.dma_gather(
                        m[:],
                        pfull[h][:, :],
                        gidx_sb[:, t0 * 8 : (t0 + nt) * 8],
                        nt * 128,
                        nt * 128,
                        D,
                        single_packet=False,
                        queue_num=(2 * ci + h) % 4,
                    )
                    mb[h] = m
                sbase = ch["t0"][0]
                scnt = ch["nt"][0] + ch["nt"][1]
                s_sb = spool.tile(
                    [D, scnt * WIN], MD, tag="S", name=f"S{l}_{sbase}"
                )
                nc.sync.dma_start(
                    s_sb[:], sflat_d[:, sbase * WIN : (sbase + scnt) * WIN]
                )
                for wv, tl in ch["windows"]:
                    ap = psum_agg.tile([D, WIN], F32, tag="agg", name=f"agg{l}_{wv}")
                    for i, (hh, off, gt) in enumerate(tl):
                        nc.tensor.matmul(
                            ap[:],
                            mb[hh][:, off, :],
                            s_sb[:, (gt - sbase) * WIN : (gt - sbase + 1) * WIN],
                            start=(i == 0),
                            stop=False,
                        )
                    # self-loop term: p_local window against diag(dinv^2)
                    nc.tensor.matmul(
                        ap[:],
                        plocal[:, wv, :],
                        diag_sb[:, wv * WIN : (wv + 1) * WIN],
                        start=False,
                        stop=True,
                    )
                    nc.scalar.activation(
                        HTn[:, wv * WIN : (wv + 1) * WIN],
                        ap[:],
                        AF.Relu,
                        bias=b_sb[l][:, 0:1],
                    )
                    if not last:
                        emit_pmm(HTn, l + 1, wv, agin_n, plocal_n)
                        if wv == nA - 1:
                            emit_ag(agin_n, pfull_n, 0, l + 1)
            if not last:
                emit_ag(agin_n, pfull_n, 1, l + 1)
                agin, pfull, plocal = agin_n, pfull_n, plocal_n
            HT = HTn

        # ---- head: out = H3 @ lin_w + lin_b ----
        stage = pstage.tile([D, nwin], F32, tag="stage")
        for w in range(nwin):
            op = psum_p.tile([D, 1], F32, tag="op", name=f"op{w}", bufs=1)
            nc.tensor.matmul(
                op[:], HT[:, w * WIN : (w + 1) * WIN], linw_sb[:, :], start=True,
                stop=True,
            )
            nc.vector.tensor_scalar(
                stage[:, w : w + 1], op[:], linb_sb[:, 0:1], None, op0=ALU.add
            )
        tp = psum_p.tile([nwin, D], F32, tag="tp", bufs=1)
        nc.tensor.transpose(tp[:], stage[:], ident_sb[:])
        ov = pstage.tile([nwin, D], F32, tag="ov")
        nc.vector.tensor_copy(ov[:], tp[:])
        nc.sync.dma_start(out_d[:, :], ov[:])

    nc.compile()
    return nc


LAST = {}


def make_in_maps(inputs, per_core, cfg: Cfg):
    x = np.ascontiguousarray(np.asarray(inputs["x"], dtype=np.float32))
    Ws = [np.asarray(inputs[f"W{l}"], dtype=np.float32) for l in range(3)]
    bs = [np.asarray(inputs[f"b{l}"], dtype=np.float32) for l in range(3)]
    lin_w = np.asarray(inputs["lin_w"], dtype=np.float32)
    lin_b = np.asarray(inputs["lin_b"], dtype=np.float32)
    spad = cfg.spad
    ident = np.eye(D, dtype=np.float32)
    in_maps = []
    for c in range(NC):
        xs = x[c * cfg.shard : (c + 1) * cfg.shard]
        xT = np.zeros((D, spad), np.float32)
        xT[:, : cfg.shard] = xs.T
        im = {
            "xT": xT,
            "lin_w": lin_w.astype(np.float32).reshape(D, 1),
            "lin_b": np.full((D, 1), float(lin_b.reshape(-1)[0]), np.float32),
            "ident": ident,
            "gidx": per_core[c]["gidx"],
            "sflat": per_core[c]["sflat"],
            "diag": per_core[c]["diag"],
        }
        for l in range(3):
            im[f"W{l}"] = Ws[l]
            im[f"b{l}"] = bs[l].reshape(D, 1)
        in_maps.append(im)
    return in_maps


def kernel(**inputs):
    cfg = Cfg()
    edge_index = np.asarray(inputs["edge_index"], dtype=np.int32)
    plan, per_core, _ = preprocess(edge_index, cfg)
    nc = build_program(plan, cfg)
    in_maps = make_in_maps(inputs, per_core, cfg)

    res = run_bass_kernel_spmd(nc, in_maps, core_ids=list(range(NC)))
    LAST["res"] = res
    out = np.zeros(cfg.n, np.float32)
    for c in range(NC):
        out[c * cfg.shard : (c + 1) * cfg.shard] = res.results[c]["out"].reshape(-1)[
            : cfg.shard
        ]
    return out


# revision 4
# speedup vs baseline: 2.1670x; 1.1154x over previous
"""Distributed GCN (3x GCNConv + linear head) on 8 TRN2 NeuronCores.

Strategy (graph/data parallel, per sharding hint):
  - Nodes block-sharded across 8 cores (5000 real rows each, padded to 5120).
  - Weights replicated; per-layer: p = H @ W computed locally per 128-node
    window (node-major pc tiles kept in SBUF as `plocal`), cast to fp16 and
    DMA'd into two staging buffers agin_A (local rows 0..2559) / agin_B
    (2560..5119). Two AllGathers (A fires as soon as windows 0..19 are done,
    overlapping the rest of the previous layer's aggregation) build two
    shared tables pfull_A/pfull_B [8*2560, 128] fp16 in DRAM.
  - Edges assigned to the core owning dst; self-loop terms are NOT edges:
    they are added per window with one extra matmul against a host-built
    diagonal S (diag(dinv^2)) using the SBUF-resident plocal tiles.
  - Remaining edges are grouped by (dst window of 128 nodes, A/B table) and
    padded to 128-edge tiles. Per-edge norm dinv[src]*dinv[dst] is folded
    into host-built one-hot scatter tiles S [slot, dst-in-window] fp16.
  - Gather: batched nc.gpsimd.dma_gather (int16 idxs < 20480) pulls message
    rows M [128 slots, 128 feat] fp16 from pfull_{A,B}; aggregation is
    PSUM += M^T @ S on the PE, flushed with Relu+bias on ACT into the next
    layer's H^T. The next layer's p-matmul for window w is emitted right
    after window w's flush, so the A-half AllGather of layer l+1 runs while
    layer l's B-half windows are still aggregating.
  - Head: out = H3 @ lin_w + lin_b via PE + transpose, one [5120] f32 per
    core, host concatenates and trims padding.

Self-contained: hardcodes the problem shapes; all host-side prep derives
from the runtime edge_index only (index bookkeeping + degree).
"""

import os
from contextlib import ExitStack
from dataclasses import dataclass, field

import numpy as np

import concourse.bacc as bacc
import concourse.bass as bass
import concourse.mybir as mybir
import concourse.tile as tile
from concourse.bass_utils import run_bass_kernel_spmd

F32 = mybir.dt.float32
F16 = mybir.dt.float16
I16 = mybir.dt.int16
AF = mybir.ActivationFunctionType
ALU = mybir.AluOpType

D = 128  # feature dim (in = hid = 128)
WIN = 128  # dst nodes per aggregation window
NC = 8  # cores


@dataclass
class Cfg:
    n: int = 40000
    e: int = 640000
    shard: int = 5000  # real nodes per core
    kwin: int = 2  # windows per gather batch
    msg_dtype: object = F16

    @property
    def spad(self):  # padded shard
        return ((self.shard + WIN - 1) // WIN) * WIN

    @property
    def nwin(self):
        return self.spad // WIN

    @property
    def hrows(self):  # local rows per A/B table half
        return self.spad // 2

    @property
    def half(self):  # rows per gather table (pfull_A or pfull_B)
        return NC * self.spad // 2


@dataclass
class Plan:
    """Per-call schedule shared by all cores (static SPMD program)."""

    caps: np.ndarray  # [nwin, 2] tiles per (window, half), max over cores
    tot: int  # total tiles per layer
    chunks: list = field(default_factory=list)
    # chunks: list of dicts:
    #  {"nt": {h: ntiles}, "t0": {h: first-global-tile},
    #   "windows": [(w, [(h, off_in_half_buf, global_tile), ...])]}


def build_plan(caps: np.ndarray, cfg: Cfg) -> Plan:
    plan = Plan(caps=caps, tot=int(caps.sum()))
    t = 0
    for w0 in range(0, cfg.nwin, cfg.kwin):
        ws = list(range(w0, min(w0 + cfg.kwin, cfg.nwin)))
        ch = {"nt": {}, "t0": {}, "windows": []}
        wtiles = {w: [] for w in ws}
        for h in (0, 1):
            ch["t0"][h] = t
            off = 0
            for w in ws:
                for _ in range(int(caps[w, h])):
                    wtiles[w].append((h, off, t))
                    off += 1
                    t += 1
            ch["nt"][h] = off
        ch["windows"] = [(w, wtiles[w]) for w in ws]
        plan.chunks.append(ch)
    assert t == plan.tot
    return plan


def preprocess(edge_index: np.ndarray, cfg: Cfg):
    """Host-side index prep. Returns (plan, per_core dict arrays, dinv)."""
    n, shard, spad, hrows, nwin = cfg.n, cfg.shard, cfg.spad, cfg.hrows, cfg.nwin
    src = edge_index[0].astype(np.int64)
    dst = edge_index[1].astype(np.int64)
    deg = 1.0 + np.bincount(dst, minlength=n).astype(np.float64)
    dinv = (1.0 / np.sqrt(deg)).astype(np.float32)

    allnorm = (dinv[src] * dinv[dst]).astype(np.float32)

    core = dst // shard
    dloc = dst % shard
    w = dloc // WIN
    dwin = (dloc % WIN).astype(np.float32)
    sc = src // shard
    sloc = src % shard
    h = sloc // hrows  # 0 = table A (local rows < hrows), 1 = table B
    idx = (sc * hrows + (sloc % hrows)).astype(np.int16)
    assert NC * hrows <= 32768

    # counts per (core, window, half)
    key = (core * nwin + w) * 2 + h
    cnt = np.bincount(key, minlength=NC * nwin * 2).reshape(NC, nwin, 2)
    caps = np.ceil(cnt.max(axis=0) / 128.0).astype(np.int64)  # [nwin, 2]
    caps = np.maximum(caps, 1)
    plan = build_plan(caps, cfg)
    tot = plan.tot

    # order edges per (core, w, h) and place into padded tile stream
    order = np.lexsort((h, w, core))
    osrcidx = idx[order]
    odwin = dwin[order]
    onorm = allnorm[order]
    okey = key[order]
    starts = np.zeros(NC * nwin * 2 + 1, dtype=np.int64)
    np.cumsum(np.bincount(okey, minlength=NC * nwin * 2), out=starts[1:])

    # global slot of each (w,h) group in the tile stream
    gslot = np.zeros((nwin, 2), dtype=np.int64)
    for ch in plan.chunks:
        for wv, tl in ch["windows"]:
            firsts = {}
            for hh, _off, gt in tl:
                if hh not in firsts:
                    firsts[hh] = gt
            for hh, gt in firsts.items():
                gslot[wv, hh] = gt

    per_core = []
    for c in range(NC):
        gi = np.zeros(tot * 128, dtype=np.int16)
        dl = np.zeros(tot * 128, dtype=np.float32)
        nv = np.zeros(tot * 128, dtype=np.float32)
        for wv in range(nwin):
            for hh in (0, 1):
                k = (c * nwin + wv) * 2 + hh
                s, e_ = starts[k], starts[k + 1]
                m = e_ - s
                if m == 0:
                    continue
                base = gslot[wv, hh] * 128
                assert m <= caps[wv, hh] * 128
                gi[base : base + m] = osrcidx[s:e_]
                dl[base : base + m] = odwin[s:e_]
                nv[base : base + m] = onorm[s:e_]
        # gather idx layout: idx i -> [i%16 (+16k replicas), i//16]
        gi16 = gi.reshape(tot * 8, 16).T  # [16, tot*8]
        gi128 = np.tile(gi16, (8, 1)).copy()  # [128, tot*8]
        # host-prebuilt scatter one-hots S [tot*128, 128] -> partition-major
        sf = np.zeros((tot * 128, 128), np.float16)
        sf[np.arange(tot * 128), dl.astype(np.int64)] = nv.astype(np.float16)
        sflat = np.ascontiguousarray(
            sf.reshape(tot, 128, 128).transpose(1, 0, 2).reshape(128, tot * 128)
        )
        # self-loop diagonal S per window: diag(dinv^2) over local rows
        dg = np.zeros((128, nwin * 128), np.float16)
        for wv in range(nwin):
            rows = np.arange(wv * 128, (wv + 1) * 128) + c * shard
            val = np.where(
                np.arange(wv * 128, (wv + 1) * 128) < shard,
                (dinv[np.minimum(rows, n - 1)] ** 2),
                0.0,
            ).astype(np.float16)
            dg[np.arange(128), wv * 128 + np.arange(128)] = val
        per_core.append({"gidx": gi128, "sflat": sflat, "diag": dg})
    return plan, per_core, dinv


def emulate(x, edge_index, Ws, bs, lin_w, lin_b, cfg: Cfg, fp16=True):
    """Numpy emulation of the exact device dataflow (for validation)."""
    plan, per_core, dinv = preprocess(edge_index, cfg)
    spad, nwin, hrows = cfg.spad, cfg.nwin, cfg.hrows
    md = np.float16 if fp16 else np.float32
    H = []  # per-core feature-major H^T [128, spad]
    for c in range(NC):
        xs = x[c * cfg.shard : (c + 1) * cfg.shard]
        H.append(
            np.concatenate([xs, np.zeros((spad - cfg.shard, D), np.float32)]).T.copy()
        )
    for l in range(3):
        W, b = Ws[l], bs[l]
        pf = [np.zeros((NC * hrows, D), md) for _ in range(2)]
        plocal = []
        for c in range(NC):
            p = (H[c].T.astype(np.float32) @ W).astype(md)  # [spad, 128]
            pf[0][c * hrows : (c + 1) * hrows] = p[:hrows]
            pf[1][c * hrows : (c + 1) * hrows] = p[hrows:]
            plocal.append(p)
        Hn = []
        for c in range(NC):
            pc = per_core[c]
            HT = np.zeros((D, spad), np.float32)
            for ch in plan.chunks:
                for wv, tl in ch["windows"]:
                    acc = np.zeros((D, WIN), np.float32)
                    for hh, _off, gt in tl:
                        ii = pc["gidx"][:16, gt * 8 : gt * 8 + 8].T.reshape(-1)
                        M = pf[hh][ii.astype(np.int64)]  # [128, D]
                        S = pc["sflat"][:, gt * 128 : (gt + 1) * 128]
                        acc += M.astype(np.float32).T @ S.astype(np.float32)
                    # self-loop diag matmul
                    Mw = plocal[c][wv * 128 : (wv + 1) * 128]  # [128, D]
                    Sd = pc["diag"][:, wv * 128 : (wv + 1) * 128]
                    acc += Mw.astype(np.float32).T @ Sd.astype(np.float32)
                    HT[:, wv * WIN : (wv + 1) * WIN] = np.maximum(
                        acc + b[:, None], 0.0
                    )
            Hn.append(HT)
        H = Hn
    out = np.zeros(cfg.n, np.float32)
    for c in range(NC):
        o = H[c].T @ lin_w[:, 0] + lin_b[0]
        out[c * cfg.shard : (c + 1) * cfg.shard] = o[: cfg.shard]
    return out


def build_program(plan: Plan, cfg: Cfg):
    """Build the SPMD Bass program (same NEFF on all 8 cores)."""
    nc = bacc.Bacc("TRN2", target_bir_lowering=False, debug=False, num_devices=NC,
                   num_swdge_queues=4, dynamic_dma_scratch_size=16384)
    spad, nwin, hrows, half, tot = cfg.spad, cfg.nwin, cfg.hrows, cfg.half, plan.tot
    MD = cfg.msg_dtype
    nA = nwin // 2  # windows in table A

    xT = nc.dram_tensor("xT", [D, spad], F32, kind="ExternalInput")
    Wd = [
        nc.dram_tensor(f"W{l}", [D, D], F32, kind="ExternalInput") for l in range(3)
    ]
    bd = [
        nc.dram_tensor(f"b{l}", [D, 1], F32, kind="ExternalInput") for l in range(3)
    ]
    linw_d = nc.dram_tensor("lin_w", [D, 1], F32, kind="ExternalInput")
    linb_d = nc.dram_tensor("lin_b", [D, 1], F32, kind="ExternalInput")
    ident_d = nc.dram_tensor("ident", [D, D], F32, kind="ExternalInput")
    gidx_d = nc.dram_tensor("gidx", [D, tot * 8], I16, kind="ExternalInput")
    sflat_d = nc.dram_tensor("sflat", [D, tot * WIN], MD, kind="ExternalInput")
    diag_d = nc.dram_tensor("diag", [D, nwin * WIN], MD, kind="ExternalInput")
    out_d = nc.dram_tensor("out", [nwin, WIN], F32, kind="ExternalOutput")

    with tile.TileContext(nc) as tc, ExitStack() as stk:
        consts = stk.enter_context(tc.tile_pool(name="consts", bufs=1))
        hpool = stk.enter_context(tc.tile_pool(name="hpool", bufs=2))
        ppool = stk.enter_context(tc.tile_pool(name="ppool", bufs=2))
        mpool = stk.enter_context(tc.tile_pool(name="mpool", bufs=3))
        spool = stk.enter_context(tc.tile_pool(name="spool", bufs=3))
        pstage = stk.enter_context(tc.tile_pool(name="pstage", bufs=2))
        psum_agg = stk.enter_context(
            tc.tile_pool(name="psum_agg", bufs=4, space="PSUM")
        )
        psum_p = stk.enter_context(tc.tile_pool(name="psum_p", bufs=2, space="PSUM"))
        dram = stk.enter_context(tc.tile_pool(name="dram", bufs=2, space="DRAM"))

        def load_const(name, dr, shape, dtype):
            t = consts.tile(shape, dtype, name=name)
            nc.sync.dma_start(t[:], dr[tuple(slice(0, s) for s in shape)])
            return t

        ident_sb = load_const("ident_sb", ident_d, [D, D], F32)
        W_sb = [load_const(f"W{l}_sb", Wd[l], [D, D], F32) for l in range(3)]
        b_sb = [load_const(f"b{l}_sb", bd[l], [D, 1], F32) for l in range(3)]
        linw_sb = load_const("linw_sb", linw_d, [D, 1], F32)
        linb_sb = load_const("linb_sb", linb_d, [D, 1], F32)
        gidx_sb = load_const("gidx_sb", gidx_d, [D, tot * 8], I16)
        diag_sb = load_const("diag_sb", diag_d, [D, nwin * WIN], MD)

        def new_ptables(l):
            agin = [
                dram.tile([hrows, D], MD, tag=f"agin{ab}", name=f"agin{ab}{l}")
                for ab in "AB"
            ]
            pfull = [
                dram.tile(
                    [half, D], MD, tag=f"pfull{ab}", name=f"pfull{ab}{l}",
                    addr_space="Shared",
                )
                for ab in "AB"
            ]
            plocal = ppool.tile([D, nwin, D], MD, tag="plocal", name=f"plocal{l}")
            return agin, pfull, plocal

        def emit_pmm(HTsrc, l, w, agin, plocal):
            """p = H[:, w] @ W_l, cast fp16, into plocal + agin half."""
            pp = psum_p.tile([D, D], F32, tag="pp", name=f"pp{l}_{w}")
            nc.tensor.matmul(
                pp[:], HTsrc[:, w * WIN : (w + 1) * WIN], W_sb[l][:],
                start=True, stop=True,
            )
            nc.vector.tensor_copy(plocal[:, w, :], pp[:])
            hh, wl = (0, w) if w < nA else (1, w - nA)
            nc.sync.dma_start(
                agin[hh][wl * WIN : (wl + 1) * WIN, :], plocal[:, w, :]
            )

        def emit_ag(agin, pfull, hh, l):
            nc.gpsimd.collective_compute(
                "AllGather",
                ALU.bypass,
                replica_groups=[list(range(NC))],
                ins=[agin[hh].opt()],
                outs=[pfull[hh].opt()],
            )

        # ---- prologue: load x, p-mms for layer 0, AGs ----
        HT = hpool.tile([D, spad], F32, tag="HT", name="HT_x")
        nc.sync.dma_start(HT[:], xT[:, :])
        agin, pfull, plocal = new_ptables(0)
        for w in range(nwin):
            emit_pmm(HT, 0, w, agin, plocal)
            if w == nA - 1:
                emit_ag(agin, pfull, 0, 0)
        emit_ag(agin, pfull, 1, 0)

        for l in range(3):
            last = l == 2
            if not last:
                agin_n, pfull_n, plocal_n = new_ptables(l + 1)
            HTn = hpool.tile([D, spad], F32, tag="HT", name=f"HT{l + 1}")
            for ci, ch in enumerate(plan.chunks):
                mb = {}
                for h in (0, 1):
                    nt = ch["nt"][h]
                    if nt == 0:
                        continue
                    m = mpool.tile(
                        [D, nt, WIN], MD, tag=f"mb{h}",
                        name=f"mb{l}_{ch['t0'][h]}_{h}",
                    )
                    t0 = ch["t0"][h]
                    nc.gpsimd.dma_gather(
                        m[:],
                        pfull[h][:, :],
                        gidx_sb[:, t0 * 8 : (t0 + nt) * 8],
                        nt * 128,
                        nt * 128,
                        D,
                        single_packet=False,
                        queue_num=(2 * ci + h) % 4,
                    )
                    mb[h] = m
                sbase = ch["t0"][0]
                scnt = ch["nt"][0] + ch["nt"][1]
                s_sb = spool.tile(
                    [D, scnt * WIN], MD, tag="S", name=f"S{l}_{sbase}"
                )
                nc.sync.dma_start(
                    s_sb[:], sflat_d[:, sbase * WIN : (sbase + scnt) * WIN]
                )
                for wv, tl in ch["windows"]:
                    ap = psum_agg.tile([D, WIN], F32, tag="agg", name=f"agg{l}_{wv}")
                    for i, (hh, off, gt) in enumerate(tl):
                        nc.tensor.matmul(
                            ap[:],
                            mb[hh][:, off, :],
                            s_sb[:, (gt - sbase) * WIN : (gt - sbase + 1) * WIN],
                            start=(i == 0),
                            stop=False,
                        )
                    # self-loop term: p_local window against diag(dinv^2)
                    nc.tensor.matmul(
                        ap[:],
                        plocal[:, wv, :],
                        diag_sb[:, wv * WIN : (wv + 1) * WIN],
                        start=False,
                        stop=True,
                    )
                    nc.scalar.activation(
                        HTn[:, wv * WIN : (wv + 1) * WIN],
                        ap[:],
                        AF.Relu,
                        bias=b_sb[l][:, 0:1],
                    )
                    if not last:
                        emit_pmm(HTn, l + 1, wv, agin_n, plocal_n)
                        if wv == nA - 1:
                            emit_ag(agin_n, pfull_n, 0, l + 1)
            if not last:
                emit_ag(agin_n, pfull_n, 1, l + 1)
                agin, pfull, plocal = agin_n, pfull_n, plocal_n
            HT = HTn

        # ---- head: out = H3 @ lin_w + lin_b ----
        stage = pstage.tile([D, nwin], F32, tag="stage")
        for w in range(nwin):
            op = psum_p.tile([D, 1], F32, tag="op", name=f"op{w}", bufs=1)
            nc.tensor.matmul(
                op[:], HT[:, w * WIN : (w + 1) * WIN], linw_sb[:, :], start=True,
                stop=True,
            )
            nc.vector.tensor_scalar(
                stage[:, w : w + 1], op[:], linb_sb[:, 0:1], None, op0=ALU.add
            )
        tp = psum_p.tile([nwin, D], F32, tag="tp", bufs=1)
        nc.tensor.transpose(tp[:], stage[:], ident_sb[:])
        ov = pstage.tile([nwin, D], F32, tag="ov")
        nc.vector.tensor_copy(ov[:], tp[:])
        nc.sync.dma_start(out_d[:, :], ov[:])

    nc.compile()
    return nc


LAST = {}


def make_in_maps(inputs, per_core, cfg: Cfg):
    x = np.ascontiguousarray(np.asarray(inputs["x"], dtype=np.float32))
    Ws = [np.asarray(inputs[f"W{l}"], dtype=np.float32) for l in range(3)]
    bs = [np.asarray(inputs[f"b{l}"], dtype=np.float32) for l in range(3)]
    lin_w = np.asarray(inputs["lin_w"], dtype=np.float32)
    lin_b = np.asarray(inputs["lin_b"], dtype=np.float32)
    spad = cfg.spad
    ident = np.eye(D, dtype=np.float32)
    in_maps = []
    for c in range(NC):
        xs = x[c * cfg.shard : (c + 1) * cfg.shard]
        xT = np.zeros((D, spad), np.float32)
        xT[:, : cfg.shard] = xs.T
        im = {
            "xT": xT,
            "lin_w": lin_w.astype(np.float32).reshape(D, 1),
            "lin_b": np.full((D, 1), float(lin_b.reshape(-1)[0]), np.float32),
            "ident": ident,
            "gidx": per_core[c]["gidx"],
            "sflat": per_core[c]["sflat"],
            "diag": per_core[c]["diag"],
        }
        for l in range(3):
            im[f"W{l}"] = Ws[l]
            im[f"b{l}"] = bs[l].reshape(D, 1)
        in_maps.append(im)
    return in_maps


def kernel(**inputs):
    cfg = Cfg()
    edge_index = np.asarray(inputs["edge_index"], dtype=np.int32)
    plan, per_core, _ = preprocess(edge_index, cfg)
    nc = build_program(plan, cfg)
    in_maps = make_in_maps(inputs, per_core, cfg)

    res = run_bass_kernel_spmd(nc, in_maps, core_ids=list(range(NC)))
    LAST["res"] = res
    out = np.zeros(cfg.n, np.float32)
    for c in range(NC):
        out[c * cfg.shard : (c + 1) * cfg.shard] = res.results[c]["out"].reshape(-1)[
            : cfg.shard
        ]
    return out
